# revision 3
# baseline (speedup 1.0000x reference)
"""Multi-head causal self-attention layer on 8 Trainium2 NeuronCores.

Problem: x[2, 2048, 1024], 16 heads x 64 dim, causal softmax attention,
fp32 reference. Sharding: core c -> (batch b = c//4, head-group hg = c%4
covering 4 heads). Each core computes its 4 heads' attention and a partial
output projection y_part = ho @ Wo[rows]; the host sums the 4 partials per
batch sample (row-parallel Wo => partial-sum reduction done host-side).

Per-core kernel layout (all matmul operands bf16, fp32 PSUM accumulation):
  xT   [1024, 2048]  x[b] transposed on host (c on partitions for QKV matmuls)
  QT,KT [d, t] head-dim on partitions: QT = Wq^T @ x^T via lhsT=Wq, rhs=xT
  V    [t, d] natural layout, +ones column (PV matmul emits softmax row-sums)
  scores^T tiles [s, q] per (head, q-tile): lhsT=KT-slice, rhs=QT-slice
  exp on ScalarE with scale=1/8; causal triu mask multiply on diagonal block
  PV: out[q, 65] += P-block^T.T @ V'-block, accumulated over s-tiles in PSUM
  normalize by ones-column sums, PE-transpose ho -> hoT, then y = hoT.T @ Wo
"""

import os

import numpy as np
import ml_dtypes

BF16 = ml_dtypes.bfloat16

N_CORES = 8
B = 2
T = 2048
C = 1024
N_HEADS = 16
HEAD = 64
NH = 4          # heads per core
D4 = NH * HEAD  # 256
P = 128
NT = T // P     # 16 token tiles
NCT = C // P    # 8 contraction tiles
VW = HEAD + 1   # 65: V plus ones column

_NC_CACHE = {}


def _build_nc():
    import concourse.bacc as bacc
    import concourse.bass as bass
    import concourse.mybir as mybir
    import concourse.tile as tile
    from contextlib import ExitStack

    BF = mybir.dt.bfloat16
    F32 = mybir.dt.float32
    AF = mybir.ActivationFunctionType

    nc = bacc.Bacc("TRN2", target_bir_lowering=False, debug=False,
                   num_devices=N_CORES)

    xT_d = nc.dram_tensor("xT", [C, T], BF, kind="ExternalInput")
    wq_d = nc.dram_tensor("wq", [C, D4], BF, kind="ExternalInput")
    wk_d = nc.dram_tensor("wk", [C, D4], BF, kind="ExternalInput")
    wv_d = nc.dram_tensor("wv", [C, D4], BF, kind="ExternalInput")
    wo_d = nc.dram_tensor("wo", [D4, C], BF, kind="ExternalInput")
    y_d = nc.dram_tensor("y", [T, C], F32, kind="ExternalOutput")

    # mask[p, j] = 1 where q >= s within the diagonal block (j >= p)
    mask_np = np.triu(np.ones((P, P), dtype=np.float32)).astype(BF16)
    mask_d = nc.inline_tensor(mask_np, name="cmask")
    ident_d = nc.inline_tensor(np.eye(P, dtype=np.float32).astype(BF16),
                               name="ident")

    with tile.TileContext(nc) as tc, ExitStack() as ctx:
        consts = ctx.enter_context(tc.tile_pool(name="consts", bufs=1))
        xT_sb = consts.tile([P, NCT, T], BF, name="xT_sb")
        wq_sb = consts.tile([P, NCT, D4], BF, name="wq_sb")
        wk_sb = consts.tile([P, NCT, D4], BF, name="wk_sb")
        wv_sb = consts.tile([P, NCT, D4], BF, name="wv_sb")
        wo_sb = consts.tile([P, 2, C], BF, name="wo_sb")
        qt_sb = consts.tile([P, 2, T], BF, name="qt_sb")
        kt_sb = consts.tile([P, 2, T], BF, name="kt_sb")
        vp_sb = consts.tile([P, NT, NH, VW], BF, name="vp_sb")
        hoT_sb = consts.tile([P, 2, T], BF, name="hoT_sb")
        mask_sb = consts.tile([P, P], BF, name="mask_sb")
        id_sb = consts.tile([P, P], BF, name="id_sb")

        nc.sync.dma_start(mask_sb[:], mask_d.ap())
        nc.sync.dma_start(id_sb[:], ident_d.ap())
        xTr = xT_d.ap().rearrange("(ct p) t -> p ct t", p=P)
        for ct in range(NCT):
            nc.sync.dma_start(xT_sb[:, ct], xTr[:, ct])
        for w_d, w_sb in ((wq_d, wq_sb), (wk_d, wk_sb), (wv_d, wv_sb)):
            nc.sync.dma_start(
                w_sb[:], w_d.ap().rearrange("(ct p) d -> p ct d", p=P))
        nc.sync.dma_start(
            wo_sb[:], wo_d.ap().rearrange("(dt p) e -> p dt e", p=P))
        nc.vector.memset(vp_sb[:, :, :, HEAD:VW], 1.0)

        # ---- QKV projections ----
        with tc.tile_pool(name="qkv_ps", bufs=4, space="PSUM") as qkv_ps:
            # QT/KT: [d on partitions, t free]; lhsT = W chunk, rhs = xT chunk
            for w_sb, o_sb in ((wq_sb, qt_sb), (wk_sb, kt_sb)):
                for di in range(2):
                    for ch in range(4):
                        ps = qkv_ps.tile([P, 512], F32, name="proj_ps",
                                         tag="proj_ps")
                        for ct in range(NCT):
                            nc.tensor.matmul(
                                ps[:],
                                w_sb[:, ct, di * P:(di + 1) * P],
                                xT_sb[:, ct, ch * 512:(ch + 1) * 512],
                                start=(ct == 0), stop=(ct == NCT - 1))
                        nc.vector.tensor_copy(
                            o_sb[:, di, ch * 512:(ch + 1) * 512], ps[:])
            # V: [t on partitions, d free]; lhsT = xT chunk, rhs = Wv chunk
            for st in range(NT):
                ps = qkv_ps.tile([P, D4], F32, name="v_ps", tag="proj_ps")
                for ct in range(NCT):
                    nc.tensor.matmul(
                        ps[:],
                        xT_sb[:, ct, st * P:(st + 1) * P],
                        wv_sb[:, ct, :],
                        start=(ct == 0), stop=(ct == NCT - 1))
                nc.vector.tensor_copy(
                    vp_sb[:, st, :, 0:HEAD],
                    ps.rearrange("p (h d) -> p h d", h=NH))

        # ---- causal attention + output projection, per 128-row q tile ----
        with ExitStack() as actx:
            sc_ps = actx.enter_context(
                tc.tile_pool(name="sc_ps", bufs=2, space="PSUM"))
            pv_ps = actx.enter_context(
                tc.tile_pool(name="pv_ps", bufs=2, space="PSUM"))
            y_ps = actx.enter_context(
                tc.tile_pool(name="y_ps", bufs=2, space="PSUM"))
            pt_pool = actx.enter_context(tc.tile_pool(name="pt_pool", bufs=3))
            sm_pool = actx.enter_context(tc.tile_pool(name="sm_pool", bufs=4))
            yo_pool = actx.enter_context(tc.tile_pool(name="yo_pool", bufs=3))

            for qt in range(NT):
                wcols = (qt + 1) * P
                pv = pv_ps.tile([P, NH * VW], F32, name="pv", tag="pv")
                pvv = pv.rearrange("p (h v) -> p h v", v=VW)
                for h in range(NH):
                    di, po = h // 2, (h % 2) * HEAD
                    pt = pt_pool.tile([P, T], BF, name="pt", tag="pt")
                    halves = [(0, min(8, qt + 1))]
                    if qt >= 8:
                        halves.append((8, qt + 1))
                    for s0, s1 in halves:
                        sc = sc_ps.tile([P, 1024], F32, name="sc", tag="sc")
                        for st in range(s0, s1):
                            nc.tensor.matmul(
                                sc[:, (st - s0) * P:(st - s0 + 1) * P],
                                kt_sb[po:po + HEAD, di, st * P:(st + 1) * P],
                                qt_sb[po:po + HEAD, di, qt * P:(qt + 1) * P],
                                start=True, stop=True)
                        nc.scalar.activation(
                            pt[:, s0 * P:s1 * P], sc[:, 0:(s1 - s0) * P],
                            AF.Exp, scale=0.125)
                    nc.vector.tensor_mul(pt[:, qt * P:wcols],
                                         pt[:, qt * P:wcols], mask_sb[:])
                    for st in range(qt + 1):
                        nc.tensor.matmul(
                            pvv[:, h, :],
                            pt[:, st * P:(st + 1) * P],
                            vp_sb[:, st, h, :],
                            start=(st == 0), stop=(st == qt))

                rr = sm_pool.tile([P, NH], F32, name="rr", tag="rr")
                nc.vector.reciprocal(rr[:], pvv[:, :, HEAD])
                ho = sm_pool.tile([P, D4], BF, name="ho", tag="ho")
                for h in range(NH):
                    nc.vector.tensor_scalar_mul(
                        ho[:, h * HEAD:(h + 1) * HEAD],
                        pvv[:, h, 0:HEAD], rr[:, h:h + 1])

                tr = sc_ps.tile([P, 2, P], BF, name="tr", tag="sc")
                for di in range(2):
                    nc.tensor.transpose(tr[:, di],
                                        ho[:, di * P:(di + 1) * P], id_sb[:])
                nc.vector.tensor_copy(hoT_sb[:, :, qt * P:(qt + 1) * P], tr[:])

                for eh in range(2):
                    yp = y_ps.tile([P, 512], F32, name="yp", tag="yp")
                    for di in range(2):
                        nc.tensor.matmul(
                            yp[:],
                            hoT_sb[:, di, qt * P:(qt + 1) * P],
                            wo_sb[:, di, eh * 512:(eh + 1) * 512],
                            start=(di == 0), stop=(di == 1))
                    ysb = yo_pool.tile([P, 512], F32, name="ysb", tag="ysb")
                    nc.vector.tensor_copy(ysb[:], yp[:])
                    nc.sync.dma_start(
                        y_d.ap()[qt * P:(qt + 1) * P, eh * 512:(eh + 1) * 512],
                        ysb[:])

    nc.compile()
    return nc


def kernel(x, Wq, Wk, Wv, Wo):
    from concourse.bass_utils import run_bass_kernel_spmd

    x = np.asarray(x, dtype=np.float32)
    Wq = np.asarray(Wq, dtype=np.float32)
    Wk = np.asarray(Wk, dtype=np.float32)
    Wv = np.asarray(Wv, dtype=np.float32)
    Wo = np.asarray(Wo, dtype=np.float32)

    in_maps = []
    for c in range(N_CORES):
        b, hg = divmod(c, 4)
        cols = slice(hg * D4, (hg + 1) * D4)
        in_maps.append({
            "xT": np.ascontiguousarray(x[b].T).astype(BF16),
            "wq": np.ascontiguousarray(Wq[:, cols]).astype(BF16),
            "wk": np.ascontiguousarray(Wk[:, cols]).astype(BF16),
            "wv": np.ascontiguousarray(Wv[:, cols]).astype(BF16),
            "wo": np.ascontiguousarray(Wo[cols, :]).astype(BF16),
        })

    if "nc" not in _NC_CACHE:
        _NC_CACHE["nc"] = _build_nc()
    nc = _NC_CACHE["nc"]

    trace = os.environ.get("BASS_MHA_TRACE", "0") == "1"
    res = run_bass_kernel_spmd(nc, in_maps, list(range(N_CORES)), trace=trace)
    if trace:
        _NC_CACHE["exec_time_ns"] = res.exec_time_ns

    y = np.zeros((B, T, C), dtype=np.float32)
    for c in range(N_CORES):
        y[c // 4] += res.results[c]["y"]
    return y


# revision 9
# speedup vs baseline: 1.5178x; 1.5178x over previous
"""Multi-head causal self-attention layer on 8 Trainium2 NeuronCores.

Problem: x[2, 2048, 1024], 16 heads x 64 dim, causal softmax attention,
fp32 reference. Sharding: core c -> (batch b = c//4, head-group hg = c%4
covering 4 heads). Each core computes its 4 heads' attention and a partial
output projection y_part = ho @ Wo[rows]; the host sums the 4 partials per
batch sample (row-parallel Wo => partial-sum reduction done host-side).

Per-core kernel layout (all matmul operands bf16, fp32 PSUM accumulation):
  xT   [1024, 2048]  x[b] transposed on host (c on partitions for QKV matmuls)
  QT,KT [d, t] head-dim on partitions: QT = Wq^T @ x^T via lhsT=Wq, rhs=xT
  V    [t, d] natural layout, +ones column (PV matmul emits softmax row-sums)
  scores^T tiles [s, q] per (head, q-tile): lhsT=KT-slice, rhs=QT-slice
  exp on ScalarE with scale=1/8; causal triu mask multiply on diagonal block
  PV: out[q, 65] += P-block^T.T @ V'-block, accumulated over s-tiles in PSUM
  normalize by ones-column sums, PE-transpose ho -> hoT, then y = hoT.T @ Wo
"""

import os

import numpy as np
import ml_dtypes

BF16 = ml_dtypes.bfloat16

N_CORES = 8
B = 2
T = 2048
C = 1024
N_HEADS = 16
HEAD = 64
NH = 4          # heads per core
D4 = NH * HEAD  # 256
P = 128
NT = T // P     # 16 token tiles
NCT = C // P    # 8 contraction tiles
VW = HEAD + 1   # 65: V plus ones column

_NC_CACHE = {}


def _build_nc():
    import concourse.bacc as bacc
    import concourse.bass as bass
    import concourse.mybir as mybir
    import concourse.tile as tile
    from contextlib import ExitStack

    BF = mybir.dt.bfloat16
    F32 = mybir.dt.float32
    AF = mybir.ActivationFunctionType

    nc = bacc.Bacc("TRN2", target_bir_lowering=False, debug=False,
                   num_devices=N_CORES)

    xT_d = nc.dram_tensor("xT", [C, T], BF, kind="ExternalInput")
    wq_d = nc.dram_tensor("wq", [C, D4], BF, kind="ExternalInput")
    wk_d = nc.dram_tensor("wk", [C, D4], BF, kind="ExternalInput")
    wv_d = nc.dram_tensor("wv", [C, D4], BF, kind="ExternalInput")
    wo_d = nc.dram_tensor("wo", [D4, C], BF, kind="ExternalInput")
    y_d = nc.dram_tensor("y", [T, C], F32, kind="ExternalOutput")

    # mask[p, j] = 1 where q >= s within the diagonal block (j >= p)
    mask_np = np.triu(np.ones((P, P), dtype=np.float32)).astype(BF16)
    mask_d = nc.inline_tensor(mask_np, name="cmask")
    ident_d = nc.inline_tensor(np.eye(P, dtype=np.float32).astype(BF16),
                               name="ident")

    with tile.TileContext(nc) as tc, ExitStack() as ctx:
        consts = ctx.enter_context(tc.tile_pool(name="consts", bufs=1))
        xT_sb = consts.tile([P, NCT, T], BF, name="xT_sb")
        wq_sb = consts.tile([P, NCT, D4], BF, name="wq_sb")
        wk_sb = consts.tile([P, NCT, D4], BF, name="wk_sb")
        wv_sb = consts.tile([P, NCT, D4], BF, name="wv_sb")
        wo_sb = consts.tile([P, 2, C], BF, name="wo_sb")
        qt_sb = consts.tile([P, 2, T], BF, name="qt_sb")
        kt_sb = consts.tile([P, 2, T], BF, name="kt_sb")
        vp_sb = consts.tile([P, NT, NH, VW], BF, name="vp_sb")
        hoT_sb = consts.tile([P, 2, T], BF, name="hoT_sb")
        mask_sb = consts.tile([P, P], BF, name="mask_sb")
        id_sb = consts.tile([P, P], BF, name="id_sb")

        nc.sync.dma_start(mask_sb[:], mask_d.ap())
        nc.sync.dma_start(id_sb[:], ident_d.ap())
        for w_d, w_sb in ((wq_d, wq_sb), (wk_d, wk_sb), (wv_d, wv_sb)):
            nc.sync.dma_start(
                w_sb[:], w_d.ap().rearrange("(ct p) d -> p ct d", p=P))
        xTr = xT_d.ap().rearrange("(ct p) t -> p ct t", p=P)
        for ct in range(NCT):
            nc.sync.dma_start(xT_sb[:, ct], xTr[:, ct])
        nc.sync.dma_start(
            wo_sb[:], wo_d.ap().rearrange("(dt p) e -> p dt e", p=P))
        nc.vector.memset(vp_sb[:, :, :, HEAD:VW], 1.0)

        # ---- QKV projections ----
        with tc.tile_pool(name="qkv_ps", bufs=4, space="PSUM") as qkv_ps:
            # QT/KT: [d on partitions, t free]; lhsT = W chunk, rhs = xT chunk
            for w_sb, o_sb in ((wq_sb, qt_sb), (wk_sb, kt_sb)):
                for di in range(2):
                    for ch in range(4):
                        ps = qkv_ps.tile([P, 512], F32, name="proj_ps",
                                         tag="proj_ps")
                        for ct in range(NCT):
                            nc.tensor.matmul(
                                ps[:],
                                w_sb[:, ct, di * P:(di + 1) * P],
                                xT_sb[:, ct, ch * 512:(ch + 1) * 512],
                                start=(ct == 0), stop=(ct == NCT - 1))
                        nc.vector.tensor_copy(
                            o_sb[:, di, ch * 512:(ch + 1) * 512], ps[:])
            # V: [t on partitions, d free]; lhsT = xT chunk, rhs = Wv chunk
            for st in range(NT):
                ps = qkv_ps.tile([P, D4], F32, name="v_ps", tag="proj_ps")
                for ct in range(NCT):
                    nc.tensor.matmul(
                        ps[:],
                        xT_sb[:, ct, st * P:(st + 1) * P],
                        wv_sb[:, ct, :],
                        start=(ct == 0), stop=(ct == NCT - 1))
                nc.vector.tensor_copy(
                    vp_sb[:, st, :, 0:HEAD],
                    ps.rearrange("p (h d) -> p h d", h=NH))

        # ---- causal attention + output projection ----
        # q processed in groups of 256 (two 128-row q tiles j=0,1 per group).
        # scores^T MMs are N=256 wide; exp batched over up to 4 s-blocks.
        with ExitStack() as actx:
            sc_ps = actx.enter_context(
                tc.tile_pool(name="sc_ps", bufs=2, space="PSUM"))
            pv_ps = actx.enter_context(
                tc.tile_pool(name="pv_ps", bufs=2, space="PSUM"))
            y_ps = actx.enter_context(
                tc.tile_pool(name="y_ps", bufs=2, space="PSUM"))
            pt_pool = actx.enter_context(tc.tile_pool(name="pt_pool", bufs=6))
            sm_pool = actx.enter_context(tc.tile_pool(name="sm_pool", bufs=4))
            yo_pool = actx.enter_context(tc.tile_pool(name="yo_pool", bufs=3))

            for g in range(NT // 2):
                q0 = 2 * g * P          # group q offset (256 wide)
                ns = 2 * g + 2          # s-blocks covering q <= group end
                pts = {}                # head -> pt sbuf tile [P, ns, 256]
                for h in range(NH):
                    di, po = h // 2, (h % 2) * HEAD
                    pt = pt_pool.tile([P, ns, 256], BF, name="pt", tag="pt",
                                      padded_shape=[P, NT, 256])
                    pts[h] = pt
                    for k in range(0, ns, 4):
                        ke = min(k + 4, ns)
                        sc = sc_ps.tile([P, 4, 256], F32, name="sc", tag="sc")
                        for st in range(k, ke):
                            nc.tensor.matmul(
                                sc[:, st - k, :],
                                kt_sb[po:po + HEAD, di, st * P:(st + 1) * P],
                                qt_sb[po:po + HEAD, di, q0:q0 + 256],
                                start=True, stop=True)
                        nc.scalar.activation(
                            pt[:, k:ke, :], sc[:, 0:ke - k, :],
                            AF.Exp, scale=0.125)
                    # causal mask on the two diagonal s-blocks
                    nc.vector.tensor_mul(pt[:, ns - 2, 0:P],
                                         pt[:, ns - 2, 0:P], mask_sb[:])
                    nc.vector.tensor_mul(pt[:, ns - 1, P:256],
                                         pt[:, ns - 1, P:256], mask_sb[:])

                for j in range(2):
                    last = ns - 2 + j   # last valid s-block for this q tile
                    qt = 2 * g + j
                    pv = pv_ps.tile([P, NH * VW], F32, name="pv", tag="pv")
                    pvv = pv.rearrange("p (h v) -> p h v", v=VW)
                    for h in range(NH):
                        for st in range(last + 1):
                            nc.tensor.matmul(
                                pvv[:, h, :],
                                pts[h][:, st, j * P:(j + 1) * P],
                                vp_sb[:, st, h, :],
                                start=(st == 0), stop=(st == last))

                    rr = sm_pool.tile([P, NH], F32, name="rr", tag="rr")
                    nc.vector.reciprocal(rr[:], pvv[:, :, HEAD])
                    ho = sm_pool.tile([P, D4], BF, name="ho", tag="ho")
                    for h in range(NH):
                        nc.vector.tensor_scalar_mul(
                            ho[:, h * HEAD:(h + 1) * HEAD],
                            pvv[:, h, 0:HEAD], rr[:, h:h + 1])

                    tr = y_ps.tile([P, 2, P], BF, name="tr", tag="yp")
                    for di in range(2):
                        nc.tensor.transpose(
                            tr[:, di], ho[:, di * P:(di + 1) * P], id_sb[:])
                    nc.vector.tensor_copy(
                        hoT_sb[:, :, qt * P:(qt + 1) * P], tr[:])

                    for eh in range(2):
                        yp = y_ps.tile([P, 512], F32, name="yp", tag="yp")
                        for di in range(2):
                            nc.tensor.matmul(
                                yp[:],
                                hoT_sb[:, di, qt * P:(qt + 1) * P],
                                wo_sb[:, di, eh * 512:(eh + 1) * 512],
                                start=(di == 0), stop=(di == 1))
                        ysb = yo_pool.tile([P, 512], F32, name="ysb", tag="ysb")
                        nc.vector.tensor_copy(ysb[:], yp[:])
                        nc.sync.dma_start(
                            y_d.ap()[qt * P:(qt + 1) * P,
                                     eh * 512:(eh + 1) * 512],
                            ysb[:])

    nc.compile()
    return nc


def kernel(x, Wq, Wk, Wv, Wo):
    from concourse.bass_utils import run_bass_kernel_spmd

    x = np.asarray(x, dtype=np.float32)
    Wq = np.asarray(Wq, dtype=np.float32)
    Wk = np.asarray(Wk, dtype=np.float32)
    Wv = np.asarray(Wv, dtype=np.float32)
    Wo = np.asarray(Wo, dtype=np.float32)

    in_maps = []
    for c in range(N_CORES):
        b, hg = divmod(c, 4)
        cols = slice(hg * D4, (hg + 1) * D4)
        in_maps.append({
            "xT": np.ascontiguousarray(x[b].T).astype(BF16),
            "wq": np.ascontiguousarray(Wq[:, cols]).astype(BF16),
            "wk": np.ascontiguousarray(Wk[:, cols]).astype(BF16),
            "wv": np.ascontiguousarray(Wv[:, cols]).astype(BF16),
            "wo": np.ascontiguousarray(Wo[cols, :]).astype(BF16),
        })

    if "nc" not in _NC_CACHE:
        _NC_CACHE["nc"] = _build_nc()
    nc = _NC_CACHE["nc"]

    trace = os.environ.get("BASS_MHA_TRACE", "0") == "1"
    res = run_bass_kernel_spmd(nc, in_maps, list(range(N_CORES)), trace=trace)
    if trace:
        _NC_CACHE["exec_time_ns"] = res.exec_time_ns
        _NC_CACHE["res"] = res

    y = np.zeros((B, T, C), dtype=np.float32)
    for c in range(N_CORES):
        y[c // 4] += res.results[c]["y"]
    return y
